# revision 20
# baseline (speedup 1.0000x reference)
"""Distributed multi-head attention kernel for 8 Trainium2 NeuronCores.

Problem: x[2,2048,768] @ Wqkv[768,2304] + bqkv -> 12-head attention -> @ Wproj + bproj.

Sharding: batch (2) x head-group (4 groups of 3 heads) = 8 cores.
Each core computes Q/K/V for its 3 heads over the full 2048-row batch,
attention for those heads, and a PARTIAL projection y_g = ctx_g @ Wproj[rows g]
(projection is linear in the ctx d-dims, so the 4 per-group partials sum
exactly). The host sums the 4 partials per batch -- no on-device collective.

Score matmuls exploit PE row-group tiling: a K=64 matmul costs the same as
K=128 (HW-verified), and two K=64 matmuls in disjoint row groups (base
partition 0 / 64) run CONCURRENTLY (~229ns per pair at N=512 vs 216ns for
one). So heads 0,1 of the group score simultaneously from the naturally
packed Q^T/K^T tiles -- no zero-padding, no memsets, ~1.9x on score cost.
Head 2 scores alone at base 0 (full rate).

The attention-value matmul reads a 128-wide window of the packed
[V_0|1|V_1|1|V_2|1] buffer; the ones column lands the softmax denominator in
the same PSUM tile (row 64 for even heads, row 63 for odd). Softmax runs
without max-subtraction (scores are O(1) for this data regime) and
normalizes late: each head's denominator row is staged to SBUF, DMA'd to
partition 0, approx-reciprocal'd, broadcast via an ones-row matmul, and
multiplied into ctx^T before that chunk's projection. Projection contracts
the group's 192 ctx dims as 2 c-tiles; c-tile 1 row 64 is an ones-row x
bproj-row pair (bias for free, on group 0 only).

Schedule: per 512-query chunk, a pair-unit (heads 0,1) then a lone-unit
(head 2) emit exp-paced score groups; every score matmul is followed by one
context matmul popped from a FIFO of pending per-head context jobs (48
score MMs : 48 ctx MMs per chunk, 1:1). QKV runs chunk-gated on the
incoming xT DMA; per-head normalize chains start inside the context jobs,
and each chunk's projection+output-DMA weaves ~1.5 units later.
"""

import numpy as np
import ml_dtypes

B = 2
L = 2048
D = 768
H = 12
HD = 64
SCALE = HD ** -0.5
N_CORES = 8
GH = 3            # heads per core
IC = 4            # query chunks of 512
ICW = L // IC     # 512

_CACHED = {}


def _build_nc():
    import concourse.bass as bass
    import concourse.mybir as mybir
    import concourse.tile as tile
    from concourse import bacc

    F32 = mybir.dt.float32
    BF16 = mybir.dt.bfloat16
    Alu = mybir.AluOpType
    Act = mybir.ActivationFunctionType

    nc = bacc.Bacc(target_bir_lowering=False)

    xT_h = nc.declare_dram_parameter("xT", [D, L], BF16, isOutput=False)
    wqkv_h = nc.declare_dram_parameter("wqkv", [128, D // 128, 576], BF16, isOutput=False)
    bqk_h = nc.declare_dram_parameter("bqk", [128, 4], F32, isOutput=False)
    bv_h = nc.declare_dram_parameter("bv", [192], F32, isOutput=False)
    wp_h = nc.declare_dram_parameter("wproj2", [128, 2, D], BF16, isOutput=False)
    y_h = nc.declare_dram_parameter("y", [L, D], BF16, isOutput=True)

    DT = D // 128      # 6 tiles of the qkv contraction dim
    LT = L // 128      # 16 key tiles
    JG = 2             # j-tiles per exp group (psum banks per score tile)
    VW = 65            # V block width per head (64 ctx + 1 ones)
    VPAD = 2 * VW + 128 + 4  # V free width: head2 window needs cols 130..258

    with tile.TileContext(nc) as tc:
        with tc.tile_pool(name="persist", bufs=1) as pp:
            KT0_sb = pp.tile([128, L], BF16)           # K^T heads 0,1 packed
            KT1_sb = pp.tile([64, L], BF16)            # K^T head 2
            QT_sb = pp.tile([128, L], BF16)            # Q^T heads 0,1 packed
            QT2_sb = pp.tile([64, L], BF16)            # Q^T head 2
            V_sb = pp.tile([128, LT, VPAD], BF16)      # [V_0|1|V_1|1|V_2|1] blocks
            OT2_sb = pp.tile([128, 2, L], BF16)        # ctx^T c-tiles (t1 row64=ones)
            bqk_sb = pp.tile([128, 4], F32)
            bv_sb = pp.tile([128, 192], F32)
            sel_sb = pp.tile([128, 128], BF16)         # ones-row bcast selector
            dst_sb = pp.tile([128, ICW], F32)          # denom staging rows 63/64
            Dh_sb = [pp.tile([1, ICW], F32, name=f"Dh{h}") for h in range(GH)]
            Rh_sb = [pp.tile([1, ICW], F32, name=f"Rh{h}") for h in range(GH)]
            R16_sb = [pp.tile([128, ICW], BF16, name=f"R16h{h}") for h in range(GH)]

            # constants
            nc.vector.memset(sel_sb, 0.0)
            nc.vector.memset(sel_sb[0:1, :], 1.0)
            for h in range(GH):
                nc.gpsimd.memset(R16_sb[h], 0.0)
                nc.vector.memset(V_sb[:, :, h * VW + HD:h * VW + HD + 1], 1.0)
            nc.vector.memset(OT2_sb[64:128, 1, :], 0.0)
            nc.vector.memset(OT2_sb[64:65, 1, :], 1.0)  # proj bias ones-row

            nc.sync.dma_start(out=bqk_sb, in_=bqk_h[:])
            bv_src = bv_h[:]
            nc.gpsimd.dma_start(
                out=bv_sb,
                in_=bass.AP(tensor=bv_src.tensor, offset=bv_src.offset,
                            ap=[[0, 128]] + list(bv_src.ap)),
            )
            with (
                tc.tile_pool(name="loadp", bufs=1) as lp,
                tc.tile_pool(name="ps_s", bufs=3, space="PSUM") as ps_s,
                tc.tile_pool(name="ps_o", bufs=2, space="PSUM") as ps_o,
                tc.tile_pool(name="ptp", bufs=2) as ptp,
                tc.tile_pool(name="ptp2", bufs=2) as ptp2,
                tc.tile_pool(name="yp", bufs=2) as yp,
            ):
                xT_sb = lp.tile([128, DT, L], BF16)
                wqkv_sb = lp.tile([128, DT, 576], BF16)
                wp_sb = lp.tile([128, 2, D], BF16)

                # critical-path DMAs first: QK weight slices + xT chunk 0
                # gate the first matmuls; V weights / wproj follow
                xT_r = xT_h[:].rearrange("(n p) l -> p n l", p=128)
                for dt in range(DT):
                    nc.sync.dma_start(
                        out=wqkv_sb[:, dt, 0:384], in_=wqkv_h[:, dt, 0:384])
                for dt in range(DT):
                    nc.sync.dma_start(
                        out=xT_sb[:, dt, 0:ICW], in_=xT_r[:, dt, 0:ICW])
                for c in range(1, IC):
                    for dt in range(DT):
                        nc.sync.dma_start(
                            out=xT_sb[:, dt, c * ICW:(c + 1) * ICW],
                            in_=xT_r[:, dt, c * ICW:(c + 1) * ICW])
                for dt in range(DT):
                    nc.sync.dma_start(
                        out=wqkv_sb[:, dt, 384:576], in_=wqkv_h[:, dt, 384:576])
                nc.sync.dma_start(out=wp_sb, in_=wp_h[:])

                # wqkv cols: [Q01 128 | K01 128 | Q2 64 | K2 64 | V 192]
                def qk_chunk(c):
                    cs = slice(c * ICW, (c + 1) * ICW)
                    ps = ps_s.tile([128, JG, ICW], F32, tag="sps")
                    for dt in range(DT):
                        nc.tensor.matmul(
                            ps[:, 0, :], wqkv_sb[:, dt, 128:256],
                            xT_sb[:, dt, cs],
                            start=(dt == 0), stop=(dt == DT - 1))
                    for dt in range(DT):
                        nc.tensor.matmul(
                            ps[:, 1, :], wqkv_sb[:, dt, 0:128],
                            xT_sb[:, dt, cs],
                            start=(dt == 0), stop=(dt == DT - 1))
                    nc.vector.tensor_scalar_add(
                        KT0_sb[:, cs], ps[:, 0, :], bqk_sb[:, 1:2])
                    nc.vector.tensor_scalar_add(
                        QT_sb[:, cs], ps[:, 1, :], bqk_sb[:, 0:1])
                    ps = ps_s.tile([128, JG, ICW], F32, tag="sps")
                    for dt in range(DT):
                        nc.tensor.matmul(
                            ps[0:64, 0, :], wqkv_sb[:, dt, 320:384],
                            xT_sb[:, dt, cs],
                            start=(dt == 0), stop=(dt == DT - 1))
                    for dt in range(DT):
                        nc.tensor.matmul(
                            ps[0:64, 1, :], wqkv_sb[:, dt, 256:320],
                            xT_sb[:, dt, cs],
                            start=(dt == 0), stop=(dt == DT - 1))
                    nc.vector.tensor_scalar_add(
                        KT1_sb[:, cs], ps[0:64, 0, :], bqk_sb[0:64, 3:4])
                    nc.vector.tensor_scalar_add(
                        QT2_sb[:, cs], ps[0:64, 1, :], bqk_sb[0:64, 2:3])

                def v_block(lt):
                    ps = ps_o.tile([128, ICW], F32, tag="ops")
                    for dt in range(DT):
                        nc.tensor.matmul(
                            ps[:, :192],
                            xT_sb[:, dt, lt * 128:(lt + 1) * 128],
                            wqkv_sb[:, dt, 384:576],
                            start=(dt == 0), stop=(dt == DT - 1))
                    nc.vector.tensor_tensor(
                        V_sb[:, lt, 0:GH * VW].rearrange(
                            "p (h c) -> p h c", c=VW)[:, :, 0:HD],
                        ps[:, :192].rearrange("p (h d) -> p h d", h=GH),
                        bv_sb[:, :].rearrange("p (h d) -> p h d", h=GH),
                        Alu.add)

                # ---- context jobs: FIFO of generators, one matmul per
                # score-matmul weave slot; evac + per-head normalize chain ----
                ctx_queue = []

                def ctx_gen(h, ic, PT):
                    p0 = (h % 2) * 64
                    dr = 64 - (h % 2)
                    voff = h * VW - p0
                    cso = slice(ic * ICW, (ic + 1) * ICW)
                    ops = ps_o.tile([128, ICW], F32, tag="ops")
                    for jt in range(LT):
                        nc.tensor.matmul(
                            ops,
                            V_sb[:, jt, voff:voff + 128],
                            PT[:, jt, :],
                            start=(jt == 0), stop=(jt == LT - 1),
                            skip_group_check=True)
                        yield
                    ot_t, ot_r = (0, p0) if h < 2 else (1, 0)
                    nc.vector.tensor_copy(
                        OT2_sb[ot_r:ot_r + 64, ot_t, cso], ops[p0:p0 + 64, :])
                    if h % 2 == 0:
                        nc.vector.tensor_copy(dst_sb[64:65, :], ops[64:65, :])
                    else:
                        nc.vector.tensor_copy(dst_sb[32:64, :], ops[32:64, :])
                    nc.sync.dma_start(
                        out=Dh_sb[h][0:1, :], in_=dst_sb[dr:dr + 1, :])
                    nc.vector.reciprocal_approx_fast(
                        out=Rh_sb[h][0:1, :], in_=Dh_sb[h][0:1, :])
                    nc.vector.tensor_copy(R16_sb[h][0:1, :], Rh_sb[h][0:1, :])

                def ctx_step():
                    while ctx_queue:
                        try:
                            next(ctx_queue[0])
                            return
                        except StopIteration:
                            ctx_queue.pop(0)

                def normalize(h, ic):
                    # bcast 1/denom (row 0) via ones-row matmul; multiply
                    # into this head's ctx^T rows
                    cs = slice(ic * ICW, (ic + 1) * ICW)
                    p0 = (h % 2) * 64
                    ot_t, ot_r = (0, p0) if h < 2 else (1, 0)
                    rb = ps_s.tile([128, JG, ICW], F32, tag="sps")
                    nc.tensor.matmul(
                        rb[:, 0, :], sel_sb, R16_sb[h], start=True, stop=True)
                    nc.vector.tensor_tensor(
                        OT2_sb[ot_r:ot_r + 64, ot_t, cs],
                        OT2_sb[ot_r:ot_r + 64, ot_t, cs],
                        rb[ot_r:ot_r + 64, 0, :], Alu.mult)

                def proj(ic):
                    # partial projection of this 512-query chunk (contract =
                    # the group's 192 dims + ones-row x bproj-row), evac + DMA
                    y_r = y_h[:].rearrange("(n p) e -> p n e", p=128)
                    for s in range(ICW // 128):
                        i0 = ic * ICW + s * 128
                        yt = yp.tile([128, D], BF16)
                        for eh in range(2):
                            pp2 = ps_o.tile([128, ICW], F32, tag="ops")
                            for t in range(2):
                                nc.tensor.matmul(
                                    pp2[:, :384],
                                    OT2_sb[:, t, i0:i0 + 128],
                                    wp_sb[:, t, eh * 384:(eh + 1) * 384],
                                    start=(t == 0), stop=(t == 1))
                            nc.vector.tensor_copy(
                                yt[:, eh * 384:(eh + 1) * 384], pp2[:, :384])
                        nc.sync.dma_start(out=y_r[:, ic * 4 + s, :], in_=yt)

                def tri_block(ic, first=False):
                    # all 3 heads per j-tile: heads 0,1 share one psum tile
                    # (row groups 0/64, SAME free-on-exp event -> they issue
                    # back-to-back and run concurrently); head 2 gets 2-jt
                    # groups. ctx weave slots go BEFORE the score pair so
                    # exp-wait is absorbed by always-ready ctx work.
                    cs = slice(ic * ICW, (ic + 1) * ICW)
                    PTc = ptp.tile([128, LT, 2 * ICW], BF16, tag="PTc")
                    PT2 = ptp2.tile([128, LT, ICW], BF16, tag="PT2")
                    for g in range(LT // JG):
                        if first and g in (2, 4, 6):
                            qk_chunk(g // 2)
                        if ic >= 1 and g == 5:
                            normalize(0, ic - 1)
                        if ic >= 1 and g == 6:
                            normalize(1, ic - 1)
                        for t in range(JG):
                            jt = JG * g + t
                            jc = slice(jt * 128, (jt + 1) * 128)
                            ctx_step()
                            ctx_step()
                            ctx_step()
                            sps = ps_s.tile([128, JG, ICW], F32, tag="sps")
                            nc.tensor.matmul(
                                sps[:, 0, :], KT0_sb[0:64, jc],
                                QT_sb[0:64, cs], start=True, stop=True)
                            nc.tensor.matmul(
                                sps[:, 1, :], KT0_sb[64:128, jc],
                                QT_sb[64:128, cs], start=True, stop=True)
                            nc.scalar.activation(
                                PTc[:, jt, :].rearrange(
                                    "p (a b) -> p a b", a=JG),
                                sps, Act.Exp, scale=SCALE)
                        spsC = ps_s.tile([128, JG, ICW], F32, tag="sps")
                        for t in range(JG):
                            jt = JG * g + t
                            jc = slice(jt * 128, (jt + 1) * 128)
                            ctx_step()
                            nc.tensor.matmul(
                                spsC[:, t, :], KT1_sb[:, jc],
                                QT2_sb[:, cs], start=True, stop=True)
                        nc.scalar.activation(
                            PT2[:, JG * g:JG * (g + 1), :], spsC, Act.Exp,
                            scale=SCALE)
                    return PTc, PT2

                # ---- schedule ----
                qk_chunk(0)
                for ic in range(IC):
                    PTc, PT2 = tri_block(ic, first=(ic == 0))
                    ctx_queue.append(ctx_gen(0, ic, PTc[:, :, 0:ICW]))
                    ctx_queue.append(ctx_gen(1, ic, PTc[:, :, ICW:2 * ICW]))
                    ctx_queue.append(ctx_gen(2, ic, PT2))
                    if ic == 0:
                        for lt in range(LT):
                            v_block(lt)
                    if ic >= 1:
                        normalize(2, ic - 1)
                        proj(ic - 1)
                while ctx_queue:
                    ctx_step()
                for h in range(GH):
                    normalize(h, IC - 1)
                proj(IC - 1)

    nc.finalize()
    return nc


def _get_nc():
    if "nc" not in _CACHED:
        _CACHED["nc"] = _build_nc()
    return _CACHED["nc"]


def _make_in_maps(x, Wqkv, bqkv, Wproj, bproj):
    bf16 = ml_dtypes.bfloat16
    x = np.asarray(x, dtype=np.float32)
    Wqkv = np.asarray(Wqkv, dtype=np.float32)
    bqkv = np.asarray(bqkv, dtype=np.float32)
    Wproj = np.asarray(Wproj, dtype=np.float32)
    bproj = np.asarray(bproj, dtype=np.float32)

    xT = [np.ascontiguousarray(x[b].T.astype(bf16)) for b in range(B)]

    in_maps = []
    for c in range(N_CORES):
        b, g = c // 4, c % 4
        q0 = 192 * g
        k0 = D + 192 * g
        v0 = 2 * D + 192 * g
        wslice = np.concatenate([
            Wqkv[:, q0:q0 + 128], Wqkv[:, k0:k0 + 128],
            Wqkv[:, q0 + 128:q0 + 192], Wqkv[:, k0 + 128:k0 + 192],
            Wqkv[:, v0:v0 + 192]], axis=1)
        wq2 = np.ascontiguousarray(
            wslice.astype(bf16).reshape(D // 128, 128, 576).transpose(1, 0, 2))
        bqk = np.zeros((128, 4), np.float32)
        bqk[:, 0] = bqkv[q0:q0 + 128]
        bqk[:, 1] = bqkv[k0:k0 + 128]
        bqk[0:64, 2] = bqkv[q0 + 128:q0 + 192]
        bqk[0:64, 3] = bqkv[k0 + 128:k0 + 192]
        bv = np.ascontiguousarray(bqkv[v0:v0 + 192])
        wp2 = np.zeros((2, 128, D), np.float32)
        wp2[0] = Wproj[192 * g:192 * g + 128, :]
        wp2[1, 0:64] = Wproj[192 * g + 128:192 * g + 192, :]
        if g == 0:
            wp2[1, 64] = bproj
        wp2 = np.ascontiguousarray(wp2.transpose(1, 0, 2).astype(bf16))
        in_maps.append({
            "xT": xT[b],
            "wqkv": wq2,
            "bqk": bqk,
            "bv": bv,
            "wproj2": wp2,
        })
    return in_maps


def run(inputs, trace=False):
    """Run the SPMD kernel. Returns (full_output [2,2048,768] f32, BassKernelResults)."""
    from concourse.bass_utils import run_bass_kernel_spmd

    nc = _get_nc()
    in_maps = _make_in_maps(**inputs)
    res = run_bass_kernel_spmd(nc, in_maps, list(range(N_CORES)), trace=trace)
    out = np.zeros((B, L, D), dtype=np.float32)
    for c in range(N_CORES):
        out[c // 4] += res.results[c]["y"].astype(np.float32)
    return out, res


def kernel(**inputs) -> np.ndarray:
    return run(inputs)[0]
